# revision 6
# baseline (speedup 1.0000x reference)
"""MoELoRA forward on 8 Trainium2 NeuronCores — fp8 split-precision version.

Data-parallel over tokens (512/core). All heavy matmuls in fp8e4 with
DoubleRow perf mode (2 K-chunks of 128 per instruction at 0.5 cyc/row).
Precision: W and x are each split hi+lo into two e4m3 tensors
(x ~= x8 + xl, 32*W^T ~= W8 + Wl); the base product uses three DR passes
x8@W8 + x8@Wl + xl@W8 (the xl@Wl term is ~0.13% and dropped), giving
~bf16-level accuracy at 0.75x bf16's PE cost. The global x32 weight scale
dodges e4m3's subnormal floor (W sigma ~ 1/32) and is divided out in the
PSUM->SBUF output copies. The LoRA path (~5% of output magnitude) runs
pure fp8: gates 32-scaled into exp(scale=1/32), t = x8@(32 U2^T),
tp = e4(t_psum * gate/16), out += tp @ e4(V2) (zero-padded DR).

Measured CPU-sim accuracy of this exact scheme: ~2.9e-3 rel err (gate 2e-2).
"""

import numpy as np
import ml_dtypes

_CACHE = {}

B, S, D_IN, D_OUT, E, R = 4, 1024, 1024, 1024, 8, 16
N_CORES = 8
N_TOK = B * S
TOK = N_TOK // N_CORES        # 512 tokens/core
ER = E * R                    # 128
HB = D_IN // 128              # 8 k-chunks
HP = HB // 2                  # 4 k-pairs
NB = TOK // 128               # 4 token blocks
SW = 32.0                     # global weight scale

BF16 = ml_dtypes.bfloat16
E4 = ml_dtypes.float8_e4m3


def _build():
    import concourse.tile as tile
    import concourse.bass_isa as bass_isa
    from concourse import bacc, mybir
    from contextlib import ExitStack

    f32 = mybir.dt.float32
    bf16 = mybir.dt.bfloat16
    fp8 = mybir.dt.float8e4
    DR = mybir.MatmulPerfMode.DoubleRow
    Exp = mybir.ActivationFunctionType.Exp
    Copy = mybir.ActivationFunctionType.Copy

    nc = bacc.Bacc("TRN2", target_bir_lowering=False, debug=False,
                   num_devices=N_CORES)
    x8_d = nc.dram_tensor("x8", [D_IN, TOK], fp8, kind="ExternalInput").ap()
    xl_d = nc.dram_tensor("xl", [D_IN, TOK], fp8, kind="ExternalInput").ap()
    w8a0_d = nc.dram_tensor("w8a0", [D_IN // 2, 512], fp8, kind="ExternalInput").ap()
    w8a1_d = nc.dram_tensor("w8a1", [D_IN // 2, 512], fp8, kind="ExternalInput").ap()
    w8b_d = nc.dram_tensor("w8b", [D_IN, 512], fp8, kind="ExternalInput").ap()
    wla_d = nc.dram_tensor("wla", [D_IN, 512], fp8, kind="ExternalInput").ap()
    wlb_d = nc.dram_tensor("wlb", [D_IN, 512], fp8, kind="ExternalInput").ap()
    u28_d = nc.dram_tensor("u28", [D_IN, ER], fp8, kind="ExternalInput").ap()
    gw8_d = nc.dram_tensor("gw8", [D_IN, 2 * E], fp8, kind="ExternalInput").ap()
    v28_d = nc.dram_tensor("v28", [ER, 2 * D_OUT], fp8, kind="ExternalInput").ap()
    sel_d = nc.dram_tensor("sel", [E, 2 * ER], fp8, kind="ExternalInput").ap()
    out_d = nc.dram_tensor("out", [TOK, D_OUT], bf16, kind="ExternalOutput").ap()

    with tile.TileContext(nc) as tc, ExitStack() as ctx:
        sb = ctx.enter_context(tc.tile_pool(name="sb", bufs=1))
        ps = ctx.enter_context(tc.tile_pool(name="ps", bufs=8, space="PSUM"))
        osb = ctx.enter_context(tc.tile_pool(name="osb", bufs=4))

        x8 = sb.tile([128, HB, TOK], fp8, tag="x8")
        xl = sb.tile([128, HB, TOK], fp8, tag="xl")
        w8 = sb.tile([128, HB, D_OUT], fp8, tag="w8")
        wl = sb.tile([128, HB, D_OUT], fp8, tag="wl")
        u28 = sb.tile([128, HB, ER], fp8, tag="u28")
        gw8 = sb.tile([128, HB, 2 * E], fp8, tag="gw8")
        v28 = sb.tile([128, 2, D_OUT], fp8, tag="v28")
        selz = sb.tile([8, 2, ER], fp8, tag="selz")
        g8t = sb.tile([8, 2, TOK], fp8, tag="g8t")
        tpt = sb.tile([128, 2, TOK], fp8, tag="tpt")
        e_sb = sb.tile([8, TOK], f32, tag="e_sb")
        se_sb = sb.tile([8, TOK], f32, tag="se_sb")
        rec_sb = sb.tile([8, TOK], f32, tag="rec_sb")
        gp_sb = sb.tile([128, TOK], f32, tag="gp_sb")
        junk_sb = sb.tile([1, 512], bf16, tag="junk_sb")

        # junk first (on the otherwise-idle Pool engine) so the PE warmup
        # starts as early as possible; then zero the DR pair slots that never
        # get written (slot 1 of g8t/tpt)
        nc.gpsimd.memset(junk_sb[:], 0.0)
        nc.vector.memset(g8t[:], 0.0)
        nc.gpsimd.memset(tpt[:], 0.0)

        # ---- input DMAs (SP ring) in consumption order; the leading x8/w8
        # chunks are pair-granular so the first base matmuls start early ----
        x8r = x8_d.rearrange("(hb p) n -> p hb n", p=128)
        w8ar = w8a0_d.rearrange("(hb p) o -> p hb o", p=128)
        nc.sync.dma_start(x8[:, 0:2, :], x8r[:, 0:2, :])
        nc.sync.dma_start(w8[:, 0:2, 0:512], w8ar[:, 0:2, :])
        nc.sync.dma_start(x8[:, 2:4, :], x8r[:, 2:4, :])
        nc.sync.dma_start(w8[:, 2:4, 0:512], w8ar[:, 2:4, :])
        nc.sync.dma_start(x8[:, 4:8, :], x8r[:, 4:8, :])
        nc.sync.dma_start(gw8[:], gw8_d.rearrange("(hb p) e -> p hb e", p=128))
        nc.sync.dma_start(u28[:], u28_d.rearrange("(hb p) e -> p hb e", p=128))
        nc.sync.dma_start(w8[:, 4:8, 0:512],
                          w8a1_d.rearrange("(hb p) o -> p hb o", p=128))
        nc.sync.dma_start(xl[:], xl_d.rearrange("(hb p) n -> p hb n", p=128))
        nc.sync.dma_start(selz[:], sel_d.rearrange("e (s r) -> e s r", s=2))
        nc.sync.dma_start(wl[:, :, 0:512], wla_d.rearrange("(hb p) o -> p hb o", p=128))
        nc.sync.dma_start(v28[:], v28_d.rearrange("e (s o) -> e s o", s=2))
        nc.sync.dma_start(w8[:, :, 512:1024], w8b_d.rearrange("(hb p) o -> p hb o", p=128))
        nc.sync.dma_start(wl[:, :, 512:1024], wlb_d.rearrange("(hb p) o -> p hb o", p=128))

        # ---- PSUM banks: misc(warm+gl), t, accA0-3, junk2, gp; accB reuses ----
        misc = ps.tile([128, 512], f32, tag="ps", name="misc")
        t_ps = ps.tile([128, TOK], f32, tag="ps", name="t")
        accA = [ps.tile([128, 512], f32, tag="ps", name=f"accA{i}") for i in range(NB)]
        junk2 = ps.tile([1, 448], f32, tag="ps", name="junk2")

        def junk_fill(n):
            # keep the PE warm through a known DMA-supply gap
            for w in range(n):
                nc.tensor.matmul(junk2[:], junk_sb[0:1, 0:1], junk_sb[:, 0:448],
                                 start=(w == 0), stop=(w == n - 1),
                                 skip_group_check=True)

        # PE warmup during initial DMA dead-time
        N_WARM = 8
        for w in range(N_WARM):
            nc.tensor.matmul(misc[0:1, 0:448], junk_sb[0:1, 0:1], junk_sb[:, 0:448],
                             start=(w == 0), stop=(w == N_WARM - 1),
                             skip_group_check=True)

        def base_mm(acc, src_x, src_w, nb, oc, hp, start):
            nc.tensor.matmul(acc[nb][:],
                             src_x[:, 2 * hp:2 * hp + 2, nb * 128:(nb + 1) * 128],
                             src_w[:, 2 * hp:2 * hp + 2, oc * 512:(oc + 1) * 512],
                             start=start, stop=False,
                             perf_mode=DR, skip_group_check=True)

        def lora_mm(acc, nb, oc):
            nc.tensor.matmul(acc[nb][:],
                             tpt[:, :, nb * 128:(nb + 1) * 128],
                             v28[:, :, oc * 512:(oc + 1) * 512],
                             start=False, stop=True,
                             perf_mode=DR, skip_group_check=True)

        def store2(acc, nb0, oc, eng0, eng1, name):
            # non-final stores ride the Pool/SWDGE ring: no HWDGE contention
            # and the scalar engine's SEQ stays clear for copies
            o2 = osb.tile([128, 2, 512], bf16, tag="o", name=name)
            with nc.allow_low_precision(reason="bf16 output"):
                eng0(o2[:, 0, :], acc[nb0])
                eng1(o2[:, 1, :], acc[nb0 + 1])
            nc.gpsimd.dma_start(
                out_d.rearrange("(j p) o -> p j o", p=128)
                     [:, nb0:nb0 + 2, oc * 512:(oc + 1) * 512], o2[:])

        def dve_copy(dst, src):
            nc.vector.tensor_scalar(dst, src[:], 1.0 / SW, None,
                                    mybir.AluOpType.mult)

        def act_copy(dst, src):
            nc.scalar.activation(dst, src[:], Copy, scale=1.0 / SW)

        # ---- phase A (oc=0) leading W8 pairs run as the chunks land; gl/t
        # slot in once x8/gw8/u28 complete ----
        oc = 0
        for hp in range(2):
            for nb in range(NB):
                base_mm(accA, x8, w8, nb, oc, hp, hp == 0)
        # gate logits gl[8, n] into misc rows 0:8 (DR pairs over k-chunks)
        for hp in range(HP):
            nc.tensor.matmul(misc[0:8, :], gw8[:, 2 * hp:2 * hp + 2, 0:E],
                             x8[:, 2 * hp:2 * hp + 2, :],
                             start=(hp == 0), stop=(hp == HP - 1),
                             perf_mode=DR, skip_group_check=True)
        # softmax chain: exp(gl/32) -> sum -> 1/sum -> g8 (fp8)
        nc.scalar.activation(e_sb[:], misc[0:8, :], Exp, scale=1.0 / SW)
        nc.gpsimd.partition_all_reduce(se_sb[:], e_sb[:], channels=8,
                                       reduce_op=bass_isa.ReduceOp.add)
        nc.vector.reciprocal(rec_sb[:], se_sb[:])
        with nc.allow_low_precision(reason="fp8 gate, lora path tolerance"):
            nc.vector.tensor_tensor(g8t[:, 0, :], e_sb[:], rec_sb[:],
                                    mybir.AluOpType.mult)
        for hp in range(HP):
            nc.tensor.matmul(t_ps[:], u28[:, 2 * hp:2 * hp + 2, :],
                             x8[:, 2 * hp:2 * hp + 2, :],
                             start=(hp == 0), stop=(hp == HP - 1),
                             perf_mode=DR, skip_group_check=True)
        for hp in range(2, HP):
            for nb in range(NB):
                base_mm(accA, x8, w8, nb, oc, hp, False)
        junk_fill(2)
        for hp in range(HP):
            for nb in range(NB):
                base_mm(accA, xl, w8, nb, oc, hp, False)
        # gate expand: gp[er, n] = g8[er//16, n]/16 (zero-padded DR), then
        # gp -> SBUF (ACT), tp = t * gp -> fp8 (DVE)
        gp_ps = ps.tile([128, TOK], f32, tag="ps", name="gp")
        nc.tensor.matmul(gp_ps[:], selz[:], g8t[:], start=True, stop=True,
                         perf_mode=DR, skip_group_check=True)
        nc.scalar.copy(gp_sb[:], gp_ps[:])
        with nc.allow_low_precision(reason="fp8 tp, lora path tolerance"):
            nc.vector.tensor_tensor(tpt[:, 0, :], t_ps[:], gp_sb[:],
                                    mybir.AluOpType.mult)
        for hp in range(HP):
            for nb in range(NB):
                base_mm(accA, x8, wl, nb, oc, hp, False)
        for nb in range(NB):
            lora_mm(accA, nb, oc)
        store2(accA, 0, oc, dve_copy, act_copy, "oA01")
        store2(accA, 2, oc, dve_copy, act_copy, "oA23")

        # ---- phase B (oc=1), bank-major so the stops stagger and the
        # copy+store pipeline drains behind the PE instead of after it ----
        oc = 1
        oview = out_d.rearrange("(j p) o -> p j o", p=128)
        accB = [ps.tile([128, 512], f32, tag="ps", name=f"accB{i}")
                for i in range(NB - 1)]
        for nb in range(NB - 1):
            for hp in range(HP):
                base_mm(accB, x8, w8, nb, oc, hp, hp == 0)
            for hp in range(HP):
                base_mm(accB, xl, w8, nb, oc, hp, False)
            for hp in range(HP):
                base_mm(accB, x8, wl, nb, oc, hp, False)
            lora_mm(accB, nb, oc)
            o1 = osb.tile([128, 512], bf16, tag="o", name=f"oB{nb}")
            with nc.allow_low_precision(reason="bf16 output"):
                if nb % 2 == 0:
                    dve_copy(o1[:], accB[nb])
                else:
                    act_copy(o1[:], accB[nb])
            nc.gpsimd.dma_start(
                oview[:, nb:nb + 1, oc * 512:(oc + 1) * 512], o1[:])

        # final bank split by columns into two separate PSUM tiles (separate
        # tiles so group 2's matmuls don't serialize behind group 1's copy):
        # [0:384] stops+stores first, leaving a [128, 128] chunk as the only
        # work on the critical tail
        nb = NB - 1
        for lo, hi, eng, ring, pnm in ((0, 384, dve_copy, nc.sync, "fin0"),
                                       (384, 512, act_copy, nc.scalar, "fin1")):
            fin = ps.tile([128, hi - lo], f32, tag="ps", name=pnm)
            for hp in range(HP):
                nc.tensor.matmul(fin[:],
                                 x8[:, 2 * hp:2 * hp + 2, nb * 128:(nb + 1) * 128],
                                 w8[:, 2 * hp:2 * hp + 2, oc * 512 + lo:oc * 512 + hi],
                                 start=(hp == 0), stop=False,
                                 perf_mode=DR, skip_group_check=True)
            for src_x, src_w in ((xl, w8), (x8, wl)):
                for hp in range(HP):
                    nc.tensor.matmul(fin[:],
                                     src_x[:, 2 * hp:2 * hp + 2, nb * 128:(nb + 1) * 128],
                                     src_w[:, 2 * hp:2 * hp + 2, oc * 512 + lo:oc * 512 + hi],
                                     start=False, stop=False,
                                     perf_mode=DR, skip_group_check=True)
            nc.tensor.matmul(fin[:],
                             tpt[:, :, nb * 128:(nb + 1) * 128],
                             v28[:, :, oc * 512 + lo:oc * 512 + hi],
                             start=False, stop=True,
                             perf_mode=DR, skip_group_check=True)
            o1 = osb.tile([128, hi - lo], bf16, tag="of", name=f"oB3_{lo}")
            with nc.allow_low_precision(reason="bf16 output"):
                eng(o1[:], fin[:])
            ring.dma_start(
                oview[:, nb:nb + 1, oc * 512 + lo:oc * 512 + hi], o1[:])

    nc.compile()
    return nc


def _get_nc():
    if "nc" not in _CACHE:
        _CACHE["nc"] = _build()
    return _CACHE["nc"]


def _q8(a):
    return np.ascontiguousarray(a).astype(E4)


def _prep_in_maps(x, weight, gate_w, lora_U, lora_V):
    xt = np.ascontiguousarray(x.reshape(N_TOK, D_IN).T)      # (D_IN, N_TOK) f32
    x8 = xt.astype(E4)
    xlr = xt - x8.astype(np.float32)
    xl8 = xlr.astype(E4)

    wTs = np.ascontiguousarray(weight.T) * SW                # (D_IN, D_OUT)
    w8 = wTs.astype(E4)
    wl8 = (wTs - w8.astype(np.float32)).astype(E4)

    u2T = np.ascontiguousarray(lora_U.reshape(ER, D_IN).T) * SW
    gwT = np.ascontiguousarray(gate_w.T) * SW
    v2 = np.ascontiguousarray(lora_V.transpose(0, 2, 1).reshape(ER, D_OUT))
    v28 = np.zeros((ER, 2, D_OUT), dtype=np.float32)
    v28[:, 0, :] = v2
    sel = np.zeros((E, 2, ER), dtype=np.float32)
    sel[:, 0, :] = np.repeat(np.eye(E, dtype=np.float32), R, axis=0).T / 16.0

    common = {
        "w8a0": _q8(w8[0:512, 0:512]), "w8a1": _q8(w8[512:1024, 0:512]),
        "w8b": _q8(w8[:, 512:1024]),
        "wla": _q8(wl8[:, 0:512]), "wlb": _q8(wl8[:, 512:1024]),
        "u28": _q8(u2T),
        "gw8": _q8(np.concatenate([gwT, np.zeros_like(gwT)], axis=1)),
        "v28": _q8(v28.reshape(ER, 2 * D_OUT)),
        "sel": _q8(sel.reshape(E, 2 * ER)),
    }
    in_maps = []
    for c in range(N_CORES):
        m = dict(common)
        m["x8"] = np.ascontiguousarray(x8[:, c * TOK:(c + 1) * TOK])
        m["xl"] = np.ascontiguousarray(xl8[:, c * TOK:(c + 1) * TOK])
        in_maps.append(m)
    return in_maps


def kernel(x, weight, gate_w, lora_U, lora_V):
    from concourse import bass_utils

    x = np.asarray(x, dtype=np.float32)
    weight = np.asarray(weight, dtype=np.float32)
    gate_w = np.asarray(gate_w, dtype=np.float32)
    lora_U = np.asarray(lora_U, dtype=np.float32)
    lora_V = np.asarray(lora_V, dtype=np.float32)

    nc = _get_nc()
    in_maps = _prep_in_maps(x, weight, gate_w, lora_U, lora_V)
    res = bass_utils.run_bass_kernel_spmd(nc, in_maps, core_ids=list(range(N_CORES)))
    out = np.concatenate([np.asarray(res.results[c]["out"]) for c in range(N_CORES)],
                         axis=0)
    return out.astype(np.float32).reshape(B, S, D_OUT)


# revision 7
# speedup vs baseline: 1.0501x; 1.0501x over previous
"""MoELoRA forward on 8 Trainium2 NeuronCores — fp8 split-precision version.

Data-parallel over tokens (512/core). All heavy matmuls in fp8e4 with
DoubleRow perf mode (2 K-chunks of 128 per instruction at 0.5 cyc/row).
Precision: W and x are each split hi+lo into two e4m3 tensors
(x ~= x8 + xl, 32*W^T ~= W8 + Wl); the base product uses three DR passes
x8@W8 + x8@Wl + xl@W8 (the xl@Wl term is ~0.13% and dropped), giving
~bf16-level accuracy at 0.75x bf16's PE cost. The global x32 weight scale
dodges e4m3's subnormal floor (W sigma ~ 1/32) and is divided out in the
PSUM->SBUF output copies. The LoRA path (~5% of output magnitude) runs
pure fp8: gates 32-scaled into exp(scale=1/32), t = x8@(32 U2^T),
tp = e4(t_psum * gate/16), out += tp @ e4(V2) (zero-padded DR).

Measured CPU-sim accuracy of this exact scheme: ~2.9e-3 rel err (gate 2e-2).
"""

import numpy as np
import ml_dtypes

_CACHE = {}

B, S, D_IN, D_OUT, E, R = 4, 1024, 1024, 1024, 8, 16
N_CORES = 8
N_TOK = B * S
TOK = N_TOK // N_CORES        # 512 tokens/core
ER = E * R                    # 128
HB = D_IN // 128              # 8 k-chunks
HP = HB // 2                  # 4 k-pairs
NB = TOK // 128               # 4 token blocks
SW = 32.0                     # global weight scale

BF16 = ml_dtypes.bfloat16
E4 = ml_dtypes.float8_e4m3


def _build():
    import concourse.tile as tile
    import concourse.bass_isa as bass_isa
    from concourse import bacc, mybir
    from contextlib import ExitStack

    f32 = mybir.dt.float32
    bf16 = mybir.dt.bfloat16
    fp8 = mybir.dt.float8e4
    DR = mybir.MatmulPerfMode.DoubleRow
    Exp = mybir.ActivationFunctionType.Exp
    Copy = mybir.ActivationFunctionType.Copy

    nc = bacc.Bacc("TRN2", target_bir_lowering=False, debug=False,
                   num_devices=N_CORES)
    x8_d = nc.dram_tensor("x8", [D_IN, TOK], fp8, kind="ExternalInput").ap()
    xl_d = nc.dram_tensor("xl", [D_IN, TOK], fp8, kind="ExternalInput").ap()
    w8a0_d = nc.dram_tensor("w8a0", [D_IN // 2, 512], fp8, kind="ExternalInput").ap()
    w8a1_d = nc.dram_tensor("w8a1", [D_IN // 2, 512], fp8, kind="ExternalInput").ap()
    w8b_d = nc.dram_tensor("w8b", [D_IN, 512], fp8, kind="ExternalInput").ap()
    wla_d = nc.dram_tensor("wla", [D_IN, 512], fp8, kind="ExternalInput").ap()
    wlb_d = nc.dram_tensor("wlb", [D_IN, 512], fp8, kind="ExternalInput").ap()
    u28_d = nc.dram_tensor("u28", [D_IN, ER], fp8, kind="ExternalInput").ap()
    gw8_d = nc.dram_tensor("gw8", [D_IN, 2 * E], fp8, kind="ExternalInput").ap()
    v28_d = nc.dram_tensor("v28", [ER, 2 * D_OUT], fp8, kind="ExternalInput").ap()
    sel_d = nc.dram_tensor("sel", [E, 2 * ER], fp8, kind="ExternalInput").ap()
    out_d = nc.dram_tensor("out", [TOK, D_OUT], bf16, kind="ExternalOutput").ap()

    with tile.TileContext(nc) as tc, ExitStack() as ctx:
        sb = ctx.enter_context(tc.tile_pool(name="sb", bufs=1))
        ps = ctx.enter_context(tc.tile_pool(name="ps", bufs=8, space="PSUM"))
        osb = ctx.enter_context(tc.tile_pool(name="osb", bufs=4))

        x8 = sb.tile([128, HB, TOK], fp8, tag="x8")
        xl = sb.tile([128, HB, TOK], fp8, tag="xl")
        w8 = sb.tile([128, HB, D_OUT], fp8, tag="w8")
        wl = sb.tile([128, HB, D_OUT], fp8, tag="wl")
        u28 = sb.tile([128, HB, ER], fp8, tag="u28")
        gw8 = sb.tile([128, HB, 2 * E], fp8, tag="gw8")
        v28 = sb.tile([128, 2, D_OUT], fp8, tag="v28")
        selz = sb.tile([8, 2, ER], fp8, tag="selz")
        g8t = sb.tile([8, 2, TOK], fp8, tag="g8t")
        tpt = sb.tile([128, 2, TOK], fp8, tag="tpt")
        e_sb = sb.tile([8, TOK], f32, tag="e_sb")
        se_sb = sb.tile([8, TOK], f32, tag="se_sb")
        rec_sb = sb.tile([8, TOK], f32, tag="rec_sb")
        gp_sb = sb.tile([128, TOK], f32, tag="gp_sb")
        junk_sb = sb.tile([1, 512], bf16, tag="junk_sb")

        # junk first (on the otherwise-idle Pool engine) so the PE warmup
        # starts as early as possible; then zero the DR pair slots that never
        # get written (slot 1 of g8t/tpt)
        nc.gpsimd.memset(junk_sb[:], 0.0)
        nc.vector.memset(g8t[:], 0.0)
        nc.gpsimd.memset(tpt[:], 0.0)

        # ---- input DMAs (SP ring) in consumption order ----
        nc.sync.dma_start(x8[:], x8_d.rearrange("(hb p) n -> p hb n", p=128))
        nc.sync.dma_start(gw8[:], gw8_d.rearrange("(hb p) e -> p hb e", p=128))
        nc.sync.dma_start(w8[:, 0:4, 0:512],
                          w8a0_d.rearrange("(hb p) o -> p hb o", p=128))
        nc.sync.dma_start(u28[:], u28_d.rearrange("(hb p) e -> p hb e", p=128))
        nc.sync.dma_start(w8[:, 4:8, 0:512],
                          w8a1_d.rearrange("(hb p) o -> p hb o", p=128))
        nc.sync.dma_start(xl[:], xl_d.rearrange("(hb p) n -> p hb n", p=128))
        nc.sync.dma_start(selz[:], sel_d.rearrange("e (s r) -> e s r", s=2))
        nc.sync.dma_start(wl[:, :, 0:512], wla_d.rearrange("(hb p) o -> p hb o", p=128))
        nc.sync.dma_start(v28[:], v28_d.rearrange("e (s o) -> e s o", s=2))
        nc.sync.dma_start(w8[:, :, 512:1024], w8b_d.rearrange("(hb p) o -> p hb o", p=128))
        nc.sync.dma_start(wl[:, :, 512:1024], wlb_d.rearrange("(hb p) o -> p hb o", p=128))

        # ---- PSUM banks: misc(warm+gl), t, accA0-3, junk2, gp; accB reuses ----
        misc = ps.tile([128, 512], f32, tag="ps", name="misc")
        t_ps = ps.tile([128, TOK], f32, tag="ps", name="t")
        accA = [ps.tile([128, 512], f32, tag="ps", name=f"accA{i}") for i in range(NB)]
        junk2 = ps.tile([1, 448], f32, tag="ps", name="junk2")

        def junk_fill(n):
            # keep the PE warm through a known DMA-supply gap
            for w in range(n):
                nc.tensor.matmul(junk2[:], junk_sb[0:1, 0:1], junk_sb[:, 0:448],
                                 start=(w == 0), stop=(w == n - 1),
                                 skip_group_check=True)

        # PE warmup during initial DMA dead-time
        N_WARM = 8
        for w in range(N_WARM):
            nc.tensor.matmul(misc[0:1, 0:448], junk_sb[0:1, 0:1], junk_sb[:, 0:448],
                             start=(w == 0), stop=(w == N_WARM - 1),
                             skip_group_check=True)

        def base_mm(acc, src_x, src_w, nb, oc, hp, start):
            nc.tensor.matmul(acc[nb][:],
                             src_x[:, 2 * hp:2 * hp + 2, nb * 128:(nb + 1) * 128],
                             src_w[:, 2 * hp:2 * hp + 2, oc * 512:(oc + 1) * 512],
                             start=start, stop=False,
                             perf_mode=DR, skip_group_check=True)

        def lora_mm(acc, nb, oc):
            nc.tensor.matmul(acc[nb][:],
                             tpt[:, :, nb * 128:(nb + 1) * 128],
                             v28[:, :, oc * 512:(oc + 1) * 512],
                             start=False, stop=True,
                             perf_mode=DR, skip_group_check=True)

        def store2(acc, nb0, oc, eng0, eng1, name):
            # non-final stores ride the Pool/SWDGE ring: no HWDGE contention
            # and the scalar engine's SEQ stays clear for copies
            o2 = osb.tile([128, 2, 512], bf16, tag="o", name=name)
            with nc.allow_low_precision(reason="bf16 output"):
                eng0(o2[:, 0, :], acc[nb0])
                eng1(o2[:, 1, :], acc[nb0 + 1])
            nc.gpsimd.dma_start(
                out_d.rearrange("(j p) o -> p j o", p=128)
                     [:, nb0:nb0 + 2, oc * 512:(oc + 1) * 512], o2[:])

        def dve_copy(dst, src):
            nc.vector.tensor_scalar(dst, src[:], 1.0 / SW, None,
                                    mybir.AluOpType.mult)

        def act_copy(dst, src):
            nc.scalar.activation(dst, src[:], Copy, scale=1.0 / SW)

        # gate logits gl[8, n] into misc rows 0:8 (DR pairs over k-chunks)
        for hp in range(HP):
            nc.tensor.matmul(misc[0:8, :], gw8[:, 2 * hp:2 * hp + 2, 0:E],
                             x8[:, 2 * hp:2 * hp + 2, :],
                             start=(hp == 0), stop=(hp == HP - 1),
                             perf_mode=DR, skip_group_check=True)
        # softmax chain: exp(gl/32) -> sum -> 1/sum -> g8 (fp8)
        nc.scalar.activation(e_sb[:], misc[0:8, :], Exp, scale=1.0 / SW)
        nc.gpsimd.partition_all_reduce(se_sb[:], e_sb[:], channels=8,
                                       reduce_op=bass_isa.ReduceOp.add)
        nc.vector.reciprocal(rec_sb[:], se_sb[:])
        with nc.allow_low_precision(reason="fp8 gate, lora path tolerance"):
            nc.vector.tensor_tensor(g8t[:, 0, :], e_sb[:], rec_sb[:],
                                    mybir.AluOpType.mult)

        # ---- phase A (oc=0); t-matmuls slotted between the W8 half-groups
        # to track the DMA arrival order (w8a0, u28, w8a1) ----
        junk_fill(2)
        oc = 0
        for hp in range(2):
            for nb in range(NB):
                base_mm(accA, x8, w8, nb, oc, hp, hp == 0)
        for hp in range(HP):
            nc.tensor.matmul(t_ps[:], u28[:, 2 * hp:2 * hp + 2, :],
                             x8[:, 2 * hp:2 * hp + 2, :],
                             start=(hp == 0), stop=(hp == HP - 1),
                             perf_mode=DR, skip_group_check=True)
        for hp in range(2, HP):
            for nb in range(NB):
                base_mm(accA, x8, w8, nb, oc, hp, False)
        junk_fill(2)
        for hp in range(HP):
            for nb in range(NB):
                base_mm(accA, xl, w8, nb, oc, hp, False)
        # gate expand: gp[er, n] = g8[er//16, n]/16 (zero-padded DR), then
        # gp -> SBUF (ACT), tp = t * gp -> fp8 (DVE)
        gp_ps = ps.tile([128, TOK], f32, tag="ps", name="gp")
        nc.tensor.matmul(gp_ps[:], selz[:], g8t[:], start=True, stop=True,
                         perf_mode=DR, skip_group_check=True)
        nc.scalar.copy(gp_sb[:], gp_ps[:])
        with nc.allow_low_precision(reason="fp8 tp, lora path tolerance"):
            nc.vector.tensor_tensor(tpt[:, 0, :], t_ps[:], gp_sb[:],
                                    mybir.AluOpType.mult)
        for hp in range(HP):
            for nb in range(NB):
                base_mm(accA, x8, wl, nb, oc, hp, False)
        for nb in range(NB):
            lora_mm(accA, nb, oc)
        store2(accA, 0, oc, dve_copy, act_copy, "oA01")
        store2(accA, 2, oc, dve_copy, act_copy, "oA23")

        # ---- phase B (oc=1), bank-major so the stops stagger and the
        # copy+store pipeline drains behind the PE instead of after it ----
        oc = 1
        oview = out_d.rearrange("(j p) o -> p j o", p=128)
        accB = [ps.tile([128, 512], f32, tag="ps", name=f"accB{i}")
                for i in range(NB - 1)]
        for nb in range(NB - 1):
            for hp in range(HP):
                base_mm(accB, x8, w8, nb, oc, hp, hp == 0)
            for hp in range(HP):
                base_mm(accB, xl, w8, nb, oc, hp, False)
            for hp in range(HP):
                base_mm(accB, x8, wl, nb, oc, hp, False)
            lora_mm(accB, nb, oc)
            o1 = osb.tile([128, 512], bf16, tag="o", name=f"oB{nb}")
            with nc.allow_low_precision(reason="bf16 output"):
                if nb % 2 == 0:
                    dve_copy(o1[:], accB[nb])
                else:
                    act_copy(o1[:], accB[nb])
            nc.gpsimd.dma_start(
                oview[:, nb:nb + 1, oc * 512:(oc + 1) * 512], o1[:])

        # final bank split by columns into two separate PSUM tiles (separate
        # tiles so group 2's matmuls don't serialize behind group 1's copy):
        # [0:384] stops+stores first, leaving a [128, 128] chunk as the only
        # work on the critical tail
        nb = NB - 1
        for lo, hi, eng, ring, pnm in ((0, 384, dve_copy, nc.sync, "fin0"),
                                       (384, 512, act_copy, nc.scalar, "fin1")):
            fin = ps.tile([128, hi - lo], f32, tag="ps", name=pnm)
            for hp in range(HP):
                nc.tensor.matmul(fin[:],
                                 x8[:, 2 * hp:2 * hp + 2, nb * 128:(nb + 1) * 128],
                                 w8[:, 2 * hp:2 * hp + 2, oc * 512 + lo:oc * 512 + hi],
                                 start=(hp == 0), stop=False,
                                 perf_mode=DR, skip_group_check=True)
            for src_x, src_w in ((xl, w8), (x8, wl)):
                for hp in range(HP):
                    nc.tensor.matmul(fin[:],
                                     src_x[:, 2 * hp:2 * hp + 2, nb * 128:(nb + 1) * 128],
                                     src_w[:, 2 * hp:2 * hp + 2, oc * 512 + lo:oc * 512 + hi],
                                     start=False, stop=False,
                                     perf_mode=DR, skip_group_check=True)
            nc.tensor.matmul(fin[:],
                             tpt[:, :, nb * 128:(nb + 1) * 128],
                             v28[:, :, oc * 512 + lo:oc * 512 + hi],
                             start=False, stop=True,
                             perf_mode=DR, skip_group_check=True)
            o1 = osb.tile([128, hi - lo], bf16, tag="of", name=f"oB3_{lo}")
            with nc.allow_low_precision(reason="bf16 output"):
                eng(o1[:], fin[:])
            ring.dma_start(
                oview[:, nb:nb + 1, oc * 512 + lo:oc * 512 + hi], o1[:])

    nc.compile()
    return nc


def _get_nc():
    if "nc" not in _CACHE:
        _CACHE["nc"] = _build()
    return _CACHE["nc"]


def _q8(a):
    return np.ascontiguousarray(a).astype(E4)


def _prep_in_maps(x, weight, gate_w, lora_U, lora_V):
    xt = np.ascontiguousarray(x.reshape(N_TOK, D_IN).T)      # (D_IN, N_TOK) f32
    x8 = xt.astype(E4)
    xlr = xt - x8.astype(np.float32)
    xl8 = xlr.astype(E4)

    wTs = np.ascontiguousarray(weight.T) * SW                # (D_IN, D_OUT)
    w8 = wTs.astype(E4)
    wl8 = (wTs - w8.astype(np.float32)).astype(E4)

    u2T = np.ascontiguousarray(lora_U.reshape(ER, D_IN).T) * SW
    gwT = np.ascontiguousarray(gate_w.T) * SW
    v2 = np.ascontiguousarray(lora_V.transpose(0, 2, 1).reshape(ER, D_OUT))
    v28 = np.zeros((ER, 2, D_OUT), dtype=np.float32)
    v28[:, 0, :] = v2
    sel = np.zeros((E, 2, ER), dtype=np.float32)
    sel[:, 0, :] = np.repeat(np.eye(E, dtype=np.float32), R, axis=0).T / 16.0

    common = {
        "w8a0": _q8(w8[0:512, 0:512]), "w8a1": _q8(w8[512:1024, 0:512]),
        "w8b": _q8(w8[:, 512:1024]),
        "wla": _q8(wl8[:, 0:512]), "wlb": _q8(wl8[:, 512:1024]),
        "u28": _q8(u2T),
        "gw8": _q8(np.concatenate([gwT, np.zeros_like(gwT)], axis=1)),
        "v28": _q8(v28.reshape(ER, 2 * D_OUT)),
        "sel": _q8(sel.reshape(E, 2 * ER)),
    }
    in_maps = []
    for c in range(N_CORES):
        m = dict(common)
        m["x8"] = np.ascontiguousarray(x8[:, c * TOK:(c + 1) * TOK])
        m["xl"] = np.ascontiguousarray(xl8[:, c * TOK:(c + 1) * TOK])
        in_maps.append(m)
    return in_maps


def kernel(x, weight, gate_w, lora_U, lora_V):
    from concourse import bass_utils

    x = np.asarray(x, dtype=np.float32)
    weight = np.asarray(weight, dtype=np.float32)
    gate_w = np.asarray(gate_w, dtype=np.float32)
    lora_U = np.asarray(lora_U, dtype=np.float32)
    lora_V = np.asarray(lora_V, dtype=np.float32)

    nc = _get_nc()
    in_maps = _prep_in_maps(x, weight, gate_w, lora_U, lora_V)
    res = bass_utils.run_bass_kernel_spmd(nc, in_maps, core_ids=list(range(N_CORES)))
    out = np.concatenate([np.asarray(res.results[c]["out"]) for c in range(N_CORES)],
                         axis=0)
    return out.astype(np.float32).reshape(B, S, D_OUT)
